# revision 31
# baseline (speedup 1.0000x reference)
"""Multi-head attention Bass kernel for Trainium2, sharded over 8 NeuronCores.

Problem: x [2, 2048, 1024] fp32; W_qkv [3072, 1024]; b_qkv [3072].
  qkv = x @ W_qkv.T + b_qkv ; split into Q,K,V of 8 heads x 128 dims;
  out  = softmax(Q K^T / sqrt(128)) V, heads re-concatenated -> [2, 2048, 1024].

Sharding: 16 (batch, head) pairs over 8 cores -> each core owns one batch
slice (b = core//4) and two heads (h0 = 2*(core%4), h0+1). Each core gets
its batch's x slice [2048, 1024] plus the W^T/bias columns for its heads,
computes the projection and full non-causal attention for its two heads,
and returns [2048, 256] (the two heads' output columns). No collectives.

v2 (startup + overlap optimized):
 - Host pre-packs x^T as [128, 8, 2048] so each DMA descriptor is an 8KB
   contiguous run; x streams in as 4 ko-pair chunks that feed a chunk-gated
   K0/Q0 projection (8 PSUM banks, ko-outer) so the PE starts ~5us earlier.
 - Weights arrive as two fully-contiguous tensors: wta = [K0|Q0] columns
   (needed first), wtb = [K1|Q1|V0|V1].
 - V projection is split per head: V0 runs before attention of head 0;
   V1-proj matmuls are interleaved into head 1's attention loop as PE
   filler while the ACT engine grinds the exps (attn is ACT-paced there).
 - Scores for (head0, qb0) are emitted right after the K0/Q0 drain so the
   ACT exp stream starts as early as possible.
 - Same math as v1: scores computed transposed (S^T tiles [k, q]); exp on
   ACT (scale folded in, no max subtraction -- scores are O(1)); PV matmul
   with stationary P-chunks and a ones column appended to V so softmax
   denominators fall out of the same matmuls; final scale by 1/denom on DVE.
"""

import math
from contextlib import ExitStack

import numpy as np

import concourse.bass as bass
import concourse.tile as tile
from concourse import bacc, mybir
from concourse.bass_utils import run_bass_kernel_spmd

# Problem constants (hardcoded per the harness contract).
B = 2
S = 2048
D = 1024
H = 8
DH = 128
N_CORES = 8
HPC = 2  # heads per core
SC = S  # tokens per core (one full batch element)
SCALE = 1.0 / math.sqrt(DH)

F32 = mybir.dt.float32
BF16 = mybir.dt.bfloat16

KO = D // 128  # 8 contraction chunks
NXCH = 4  # x arrives in 4 ko-pair chunks
QB = 256  # query block width
NQB = SC // QB  # 8
NKT = S // 128  # 16 key tiles
KPS = 4  # key tiles per score/exp group (exp on [128, KPS*QB] = [128,1024])
NTB = 4  # 512-token tiles for Q/K projection
TB = SC // NTB  # 512


def _mha_body(ctx: ExitStack, tc: tile.TileContext, out, x, wtak, wtaq, wtb, bqk_d, bias_v):
    nc = tc.nc

    consts = ctx.enter_context(tc.tile_pool(name="consts", bufs=1))
    xtp = ctx.enter_context(tc.tile_pool(name="xtp", bufs=1))
    qkvp = ctx.enter_context(tc.tile_pool(name="qkvp", bufs=1))

    # hoist the ACT exp table load (~2.7us) to kernel start, under the input DMA
    warm = consts.tile([128, 1], F32)
    nc.vector.memset(warm, 0.0)
    nc.scalar.activation(warm, warm, mybir.ActivationFunctionType.Exp)

    # ---- input DMAs, ordered for earliest first matmul ----
    # sync HWDGE queue carries ONLY the x chunks (the critical path: chunk 0
    # gates the first matmul); the scalar HWDGE queue carries the weights
    # (wta first -- also needed by the first matmul -- then bias, then wtb);
    # gpsimd SWDGE: the broadcast V bias.
    # chunk splits: first two are single-ko (0.5MB) so the PE starts sooner
    xt = xtp.tile([128, KO, SC], BF16)  # [ki, ko, tok]
    # ko0 arrives in two token-half chunks (0.25MB each) so the very first
    # K-projection matmuls (tb0/tb1) are gated on as little DMA as possible
    nc.sync.dma_start(xt[:, 0:1, 0:SC // 2], x[:, 0:1, 0:SC // 2])
    nc.sync.dma_start(xt[:, 0:1, SC // 2:], x[:, 0:1, SC // 2:])
    for k0, k1 in ((1, 2), (2, 4), (4, 6), (6, 8)):
        nc.sync.dma_start(xt[:, k0:k1, :], x[:, k0:k1, :])

    wtak_sb = consts.tile([128, KO, DH], BF16)  # [ki, ko, m]: k0 cols
    nc.scalar.dma_start(wtak_sb, wtak)
    wtaq_sb = consts.tile([128, KO, DH], BF16)  # q0 cols
    nc.scalar.dma_start(wtaq_sb, wtaq)

    # wtb rides the sync ring BEHIND the x chunks: it is not consumed until
    # the V0 projection (~30us) and putting it on the scalar ring would
    # steal HBM bandwidth from the x chunks that gate phase 0.
    wtb_sb = consts.tile([128, KO, 4 * DH], BF16)  # m = [k1 q1 v0 v1]
    nc.sync.dma_start(wtb_sb, wtb)

    # Q/K biases packed host-side as [128, 4] (cols: q0 q1 k0 k1)
    bqk_sb = consts.tile([128, 2 * HPC], F32)
    nc.scalar.dma_start(bqk_sb, bqk_d)
    bqk = [bqk_sb[:, i:i + 1] for i in range(2 * HPC)]
    # V bias replicated across partitions [128, 256] (both heads)
    bv_rep = consts.tile([128, HPC * DH], F32)
    nc.gpsimd.dma_start(bv_rep, bias_v[None, :].to_broadcast([128, HPC * DH]))

    # ---- persistent QKV tiles ----
    qT = qkvp.tile([128, HPC, SC], BF16, tag="qT")  # [dh, h, tok]
    kT = qkvp.tile([128, HPC, SC], BF16, tag="kT")
    v_sb = qkvp.tile([128, HPC, SC // 128, DH + 1], BF16, tag="v")  # [tok_i, h, tok_o, dh+1]
    nc.vector.memset(v_sb[:, :, :, DH:DH + 1], 1.0)

    # ---- phase 0: K0 + Q0 projection, ko-outer (chunk-gated), 8 PSUM banks ----
    with ExitStack() as ctx0:
        p0 = ctx0.enter_context(tc.tile_pool(name="p0ps", bufs=1, space="PSUM"))
        # 8 tiles of [128, 512] f32 = 8 banks: [kq][tb]
        p0t = [[p0.tile([128, TB], F32, tag=f"p0_{kq}_{tb}", name=f"p0_{kq}_{tb}")
                for tb in range(NTB)] for kq in range(2)]

        def p0_mm(kq, tb, ko):
            wsrc = wtak_sb if kq == 0 else wtaq_sb
            nc.tensor.matmul(
                p0t[kq][tb],
                lhsT=wsrc[:, ko, :],
                rhs=xt[:, ko, tb * TB:(tb + 1) * TB],
                start=(ko == 0),
                stop=(ko == KO - 1),
            )

        for ko in range(2 * NXCH - 2):
            for kq in range(2):  # 0 = k0, 1 = q0
                for tb in range(NTB):
                    p0_mm(kq, tb, ko)
        # last chunk: tile-major, each tile's drain emitted right behind its
        # final matmul so the DVE chases the PE instead of serializing after.
        # K tiles first, then Q tb0 (so scores(0, qb0) can issue earliest),
        # then the high-bank Q tiles in reverse so whichever end of the bank
        # range the attention pools reuse is freed early.
        drain_order = [(0, tb) for tb in range(NTB)] + [(1, 0), (1, 3), (1, 2), (1, 1)]
        for i, (kq, tb) in enumerate(drain_order):
            p0_mm(kq, tb, 2 * NXCH - 2)
            p0_mm(kq, tb, 2 * NXCH - 1)
            dst = kT if kq == 0 else qT
            b = bqk[HPC] if kq == 0 else bqk[0]
            # alternate DVE / ACT so the 8 drains run in parallel pairs (the
            # ACT engine is otherwise idle until the first exp)
            if i % 2 == 0:
                nc.vector.tensor_scalar_add(dst[:, 0, tb * TB:(tb + 1) * TB], p0t[kq][tb], b)
            else:
                nc.scalar.add(dst[:, 0, tb * TB:(tb + 1) * TB], p0t[kq][tb], b)

        # first two V0 groups reuse the first two drained K banks, so they
        # start without waiting for the phase-0 pool-close WAR barrier
        # (which gates on ALL eight drains)
        for g in range(2):
            psv = p0t[0][g][:, :DH]
            for ko in range(KO):
                nc.tensor.matmul(
                    psv,
                    lhsT=xt[:, ko, g * 128:(g + 1) * 128],
                    rhs=wtb_sb[:, ko, 2 * DH:3 * DH],
                    start=(ko == 0),
                    stop=(ko == KO - 1),
                )
            nc.vector.tensor_add(v_sb[:, 0, g, 0:DH], psv, bv_rep[:, 0:DH])

    # ---- main pools (reuse phase-0 PSUM banks; Tile inserts WAR syncs) ----
    proj_ps = ctx.enter_context(tc.tile_pool(name="proj_ps", bufs=2, space="PSUM"))
    st_ps = ctx.enter_context(tc.tile_pool(name="st_ps", bufs=2, space="PSUM"))
    pv_ps = ctx.enter_context(tc.tile_pool(name="pv_ps", bufs=2, space="PSUM"))
    atp = ctx.enter_context(tc.tile_pool(name="atp", bufs=6))
    outp = ctx.enter_context(tc.tile_pool(name="outp", bufs=2))
    rcp = ctx.enter_context(tc.tile_pool(name="rcp", bufs=8))

    def emit_qk1_group(tb):
        # K1/Q1 projection for one token block (ko-inner, proj pool)
        for kq in range(2):  # 0 = k1, 1 = q1
            ps = proj_ps.tile([128, TB], F32, tag="ps", name="ps")
            for ko in range(KO):
                nc.tensor.matmul(
                    ps,
                    lhsT=wtb_sb[:, ko, kq * DH:(kq + 1) * DH],
                    rhs=xt[:, ko, tb * TB:(tb + 1) * TB],
                    start=(ko == 0),
                    stop=(ko == KO - 1),
                )
            dst = kT if kq == 0 else qT
            b = bqk[HPC + 1] if kq == 0 else bqk[1]
            nc.vector.tensor_scalar_add(dst[:, 1, tb * TB:(tb + 1) * TB], ps, b)

    def emit_v_group(h, tb):
        # V projection for head h, one 128-token tile: out [128 tok, 128]
        ps = proj_ps.tile([128, TB], F32, tag="ps", name="ps")
        psv = ps[:, :DH]
        for ko in range(KO):
            nc.tensor.matmul(
                psv,
                lhsT=xt[:, ko, tb * 128:(tb + 1) * 128],
                rhs=wtb_sb[:, ko, (2 + h) * DH:(3 + h) * DH],
                start=(ko == 0),
                stop=(ko == KO - 1),
            )
        nc.vector.tensor_add(
            v_sb[:, h, tb, 0:DH], psv, bv_rep[:, h * DH:(h + 1) * DH]
        )

    def emit_scores(h, qb):
        qs = slice(qb * QB, (qb + 1) * QB)
        ats = []
        for ktg in range(NKT // KPS):
            st = st_ps.tile([128, KPS, QB], F32, tag="st", name="st")
            for i in range(KPS):
                kt = ktg * KPS + i
                nc.tensor.matmul(
                    st[:, i, :],
                    lhsT=kT[:, h, kt * 128:(kt + 1) * 128],
                    rhs=qT[:, h, qs],
                    start=True,
                    stop=True,
                )
            at = atp.tile([128, KPS, QB], BF16, tag="at", name="at")
            nc.scalar.activation(at, st, mybir.ActivationFunctionType.Exp, scale=SCALE)
            ats.append(at)
        return ats

    def emit_pv(h, qb, ats):
        qs = slice(qb * QB, (qb + 1) * QB)
        pvs = [pv_ps.tile([128, DH + 1], F32, tag="pv", name=f"pv{j}") for j in range(QB // 128)]
        for ktg in range(NKT // KPS):
            at = ats[ktg]
            for i in range(KPS):
                kt = ktg * KPS + i
                for j in range(QB // 128):
                    nc.tensor.matmul(
                        pvs[j],
                        lhsT=at[:, i, j * 128:(j + 1) * 128],
                        rhs=v_sb[:, h, kt, :],
                        start=(kt == 0),
                        stop=(kt == NKT - 1),
                    )
        # per-j scale + DMA so each output half ships as soon as it is scaled
        # (shortens the kernel tail vs one combined DMA per qb)
        ot = outp.tile([128, QB // 128, DH], F32, tag="ot", name="ot")
        for j in range(QB // 128):
            rc = rcp.tile([128, 1], F32, tag="rc", name="rc")
            nc.vector.reciprocal(rc, pvs[j][:, DH:DH + 1])
            nc.vector.tensor_scalar_mul(ot[:, j, :], pvs[j][:, 0:DH], rc)
            nc.sync.dma_start(
                out[qb * QB + j * 128:qb * QB + (j + 1) * 128, h * DH:(h + 1) * DH],
                ot[:, j, :],
            )

    # ---- emission schedule ----
    # (V0 groups 0-1 were emitted inside phase 0, reusing drained K banks)
    # scores for (h0, qb0) as early as possible: starts the ACT exp stream
    ats00 = emit_scores(0, 0)
    for tb in range(2, SC // 128):
        emit_v_group(0, tb)
    emit_pv(0, 0, ats00)
    # attn(0) qb1..7, interleaved with K1/Q1 projection and V1 projection
    # (PE filler keeps the PE ahead of the ACT-paced exp stream)
    qk1_tbs = list(range(NTB))
    v1_tbs = list(range(SC // 128))
    for qb in range(1, NQB):
        ats = emit_scores(0, qb)
        if qk1_tbs:
            emit_qk1_group(qk1_tbs.pop(0))
        if qb >= 2:
            for _ in range(2):
                if v1_tbs:
                    emit_v_group(1, v1_tbs.pop(0))
        emit_pv(0, qb, ats)
    while v1_tbs:
        emit_v_group(1, v1_tbs.pop(0))
    # attn(1)
    for qb in range(NQB):
        ats = emit_scores(1, qb)
        emit_pv(1, qb, ats)


def build_program():
    nc = bacc.Bacc("TRN2", target_bir_lowering=False, debug=False)
    x = nc.dram_tensor("x", [128, KO, SC], BF16, kind="ExternalInput").ap()
    wtak = nc.dram_tensor("wtak", [128, KO, DH], BF16, kind="ExternalInput").ap()
    wtaq = nc.dram_tensor("wtaq", [128, KO, DH], BF16, kind="ExternalInput").ap()
    wtb = nc.dram_tensor("wtb", [128, KO, 4 * DH], BF16, kind="ExternalInput").ap()
    bqk_d = nc.dram_tensor("bqk", [128, 2 * HPC], F32, kind="ExternalInput").ap()
    bias_v = nc.dram_tensor("bias_v", [HPC * DH], F32, kind="ExternalInput").ap()
    out = nc.dram_tensor("out", [SC, HPC * DH], F32, kind="ExternalOutput").ap()
    with tile.TileContext(nc) as tc:
        with ExitStack() as ctx:
            _mha_body(ctx, tc, out, x, wtak, wtaq, wtb, bqk_d, bias_v)
    nc.compile()
    return nc


_NC = None


def _get_nc():
    global _NC
    if _NC is None:
        _NC = build_program()
    return _NC


def make_in_maps(x, W_qkv, b_qkv):
    import ml_dtypes

    x = np.asarray(x, dtype=np.float32)
    W = np.asarray(W_qkv, dtype=np.float32)
    b = np.asarray(b_qkv, dtype=np.float32)
    x_bf = x.astype(ml_dtypes.bfloat16)
    in_maps = []
    for c in range(N_CORES):
        bsel = c // 4
        h0 = HPC * (c % 4)
        # x^T as [ki=128, ko=8, tok]: element (p, ko, t) = x[bsel].T[ko*128+p, t]
        xT = np.ascontiguousarray(
            x_bf[bsel].T.reshape(KO, 128, SC).transpose(1, 0, 2)
        )
        # W rows for this core's heads: q_h at h*128, k_h at D+h*128, v_h at 2D+h*128
        def wrows(block, h):  # block: 0=q, 1=k, 2=v
            r0 = block * D + (h0 + h) * DH
            return W[r0:r0 + DH]

        # wtak = k0 cols, wtaq = q0 cols; wtb m-order: [k1 q1 v0 v1]
        wb = np.concatenate([wrows(1, 1), wrows(0, 1), wrows(2, 0), wrows(2, 1)], axis=0)

        def pack_wt(wm):  # [m, 1024] -> [ki=128, ko=8, m]
            wt = wm.T.astype(ml_dtypes.bfloat16)  # [1024, m]
            return np.ascontiguousarray(wt.reshape(KO, 128, wm.shape[0]).transpose(1, 0, 2))

        # bias host order: [q0 q1 k0 k1 v0 v1] blocks of 128
        brows = np.concatenate([
            b[(h0 + 0) * DH:(h0 + 1) * DH],
            b[(h0 + 1) * DH:(h0 + 2) * DH],
            b[D + (h0 + 0) * DH:D + (h0 + 1) * DH],
            b[D + (h0 + 1) * DH:D + (h0 + 2) * DH],
            b[2 * D + (h0 + 0) * DH:2 * D + (h0 + 1) * DH],
            b[2 * D + (h0 + 1) * DH:2 * D + (h0 + 2) * DH],
        ])
        in_maps.append(
            {
                "x": xT,
                "wtak": pack_wt(wrows(1, 0)),
                "wtaq": pack_wt(wrows(0, 0)),
                "wtb": pack_wt(wb),
                "bqk": np.ascontiguousarray(brows[:512].reshape(4, 128).T),
                "bias_v": np.ascontiguousarray(brows[512:]),
            }
        )
    return in_maps


def gather_output(results):
    outp = np.empty((B, S, D), np.float32)
    for c in range(N_CORES):
        o = results[c]["out"]
        bsel = c // 4
        h0 = HPC * (c % 4)
        outp[bsel, :, h0 * DH:(h0 + HPC) * DH] = o
    return outp


def kernel(x, W_qkv, b_qkv, **run_kwargs):
    in_maps = make_in_maps(x, W_qkv, b_qkv)
    res = run_bass_kernel_spmd(_get_nc(), in_maps, core_ids=list(range(N_CORES)), **run_kwargs)
    out = gather_output(res.results)
    if run_kwargs:
        kernel.last_result = res
    return out
